# revision 7
# baseline (speedup 1.0000x reference)
"""Multi-head self-attention (RoPE + softmax + out-proj) for Trainium2,
sharded over 8 NeuronCores: data-parallel over batch (4) x tensor-parallel
over heads (2 groups of 8). Each core computes q/k/v projections for its
head group, attention, and a partial output projection; the host sums the
two partials per batch and adds the bias.

Per-core layout highlights:
  - All matmul operands are float32r (rounded fp32), which streams at the
    full 1 cycle/row PE rate at N=512 (plain fp32 runs at 1/4 rate).
  - q/k are produced transposed ([head_dim, n]) by projecting against the
    Wq / Wkv column slices; RoPE's rotate_half is done with 32-partition
    shifted DVE copies, with the sign folded into a host-negated sin table.
  - Scores are computed transposed (S^T[m, n]) with K=64 row-group-packed
    matmul pairs (two heads concurrently in the PE array), so softmax's
    sum over keys m becomes a matmul reduction: v is extended with a ones
    column (M=65 matmul) whose output row 64 accumulates the denominator.
  - exp runs on the scalar engine straight out of PSUM in 1024-wide
    instructions; normalization uses a DVE reciprocal plus K=1 broadcast
    matmuls; the attention wave of pair p is software-pipelined with the
    projections of pair p+1 and the output projection of finished quarters
    so the tensor engine stays dense (HAM stays un-throttled).
"""

import numpy as np

import concourse.bass as bass
import concourse.mybir as mybir
import concourse.tile as tile

B, N, DIM, H, DH = 4, 2048, 1024, 16, 64
SCALE = DH**-0.5
N_CORES = 8
HG = 8  # heads per core
INNER = HG * DH  # 512, inner dim slice per core
PAIRS = INNER // 128  # 4 head pairs (=128-partition inner chunks)
NB = 4  # n blocks of 512
MB = 16  # m blocks of 128
KD = DIM // 128  # 8 contraction chunks

F32 = mybir.dt.float32
F32R = mybir.dt.float32r
EXP = mybir.ActivationFunctionType.Exp

MAX_WAITS = 1


def _split_excess_waits(nc):
    """This walrus build rejects >1 semaphore wait per instruction; hoist
    excess waits onto nops inserted before the instruction on its engine."""
    import bass_rust

    for f in nc.m.functions:
        for bb in f.blocks:
            il = bb.instructions
            i = 0
            while i < len(il):
                inst = il[i]
                si = inst.sync_info
                if si is not None and si.on_wait and len(si.on_wait) > MAX_WAITS:
                    waits = list(si.on_wait)
                    si.on_wait = waits[:MAX_WAITS]
                    rest = waits[MAX_WAITS:]
                    eng = nc.engines[inst.engine]
                    insert_at = i
                    for j in range(0, len(rest), MAX_WAITS):
                        b = eng.nop(nofuse=True, hint="wait_split")
                        ni = b.ins
                        tail = nc.cur_bb.bb.instructions
                        assert tail[-1] is ni
                        tail.pop()
                        nsi = ni.sync_info
                        if nsi is None:
                            ni.sync_info = bass_rust.SyncInfo(
                                on_wait=rest[j : j + MAX_WAITS], on_update=[]
                            )
                        else:
                            nsi.on_wait = rest[j : j + MAX_WAITS]
                        il.insert(insert_at, ni)
                        insert_at += 1
                        i += 1
                i += 1


class _FixedTileContext(tile.TileContext):
    def __exit__(self, exc_type, exc_val, exc_tb):
        res = super().__exit__(exc_type, exc_val, exc_tb)
        if exc_type is None:
            _split_excess_waits(self.nc)
        return res


def build_kernel():
    nc = bass.Bass()
    xT = nc.dram_tensor("xT", [DIM, N], F32, kind="ExternalInput")
    wq = nc.dram_tensor("wq", [DIM, INNER], F32, kind="ExternalInput")
    wk = nc.dram_tensor("wk", [DIM, INNER], F32, kind="ExternalInput")
    wv = nc.dram_tensor("wv", [DIM, INNER], F32, kind="ExternalInput")
    wo = nc.dram_tensor("wo", [INNER, DIM], F32, kind="ExternalInput")
    cosT = nc.dram_tensor("cosT", [128, N], F32, kind="ExternalInput")
    sinT = nc.dram_tensor("sinT", [128, N], F32, kind="ExternalInput")
    out = nc.dram_tensor("out", [N, DIM], F32, kind="ExternalOutput")

    vs = nc.dram_tensor("vs", [N, INNER], F32R)  # v bounce scratch
    xr = nc.dram_tensor("xr", [DIM, N], F32R)  # pre-cast x^T

    xTr = xr.rearrange("(c p) n -> p c n", p=128)

    with _FixedTileContext(nc) as tc:
        with (
            tc.tile_pool(name="const", bufs=1) as cpool,
            tc.tile_pool(name="qk", bufs=1) as qkpool,
            tc.tile_pool(name="ps", space=bass.MemorySpace.PSUM, bufs=1) as ps,
            tc.tile_pool(name="io", bufs=1) as iopool,
        ):
            # ---- constants ----
            cos_t = cpool.tile([128, N], F32, tag="cos")
            sin_t = cpool.tile([128, N], F32, tag="sin")
            nc.sync.dma_start(cos_t[:], cosT[:])
            nc.sync.dma_start(sin_t[:], sinT[:])
            ones_f = cpool.tile([128, 64], F32, tag="onesf")
            nc.vector.memset(ones_f[:], 1.0)
            onesr = cpool.tile([128, 64], F32R, tag="onesr")
            nc.vector.tensor_copy(onesr[:], ones_f[:])

            # ---- per-pair q/k projection blocks            # ---- per-pair q/k projection blocks (emitted interleaved with
            #      the previous pair's attention so the PE never idles) ----
            def proj_pair_blocks(p):
                csl = slice(p * 128, (p + 1) * 128)
                wt = {}

                def load_w():
                    for nm, wd in (("q", wq), ("k", wk)):
                        t = iopool.tile([128, KD, 128], F32R, tag=f"w{nm}", bufs=1, name=f"w{nm}_{p}")
                        nc.gpsimd.dma_start(
                            t[:], wd.rearrange("(c p) i -> p c i", p=128)[:, :, csl]
                        )
                        wt[nm] = t
                qT_t = qkpool.tile([128, N], F32R, tag="qT", bufs=2)
                kT_t = qkpool.tile([128, N], F32R, tag="kT", bufs=2)

                xts = {}

                def block(nb, pl, nm, tgt):
                    def emit():
                        if p == 0 and nb in first_x:
                            xts[nb] = first_x[nb]
                        if nb not in xts:
                            x_t = iopool.tile(
                                [128, KD, 512], F32R, tag="xv", bufs=2,
                                name=f"x_{p}_{nb}",
                            )
                            nc.sync.dma_start(
                                x_t[:], xTr[:, :, nb * 512 : (nb + 1) * 512]
                            )
                            xts[nb] = x_t
                        x_t = xts[nb]
                        nsl = slice(nb * 512, (nb + 1) * 512)
                        pq = ps.tile([128, 2, 512], F32, tag="s", bufs=3)
                        for dc in range(KD):
                            nc.tensor.matmul(
                                pq[:, 0, :], wt[nm][:, dc, :], x_t[:, dc, :],
                                start=(dc == 0), stop=(dc == KD - 1),
                            )
                        # rotate_half via 32-partition shifted copies; sign
                        # folded into sin_t (host negates low half rows)
                        tmp = iopool.tile([128, 512], F32, tag="tmp", bufs=2)
                        for g in range(4):
                            dst = slice(g * 32, (g + 1) * 32)
                            ssrc = slice((g ^ 1) * 32, ((g ^ 1) + 1) * 32)
                            nc.vector.tensor_copy(tmp[dst, :], pq[ssrc, 0, :])
                        nc.vector.tensor_mul(tmp[:], tmp[:], sin_t[:, nsl])
                        nc.vector.tensor_mul(tgt[:, nsl], pq[:, 0, :], cos_t[:, nsl])
                        nc.vector.tensor_add(tgt[:, nsl], tgt[:, nsl], tmp[:])
                    return emit

                blocks = []
                for nb in range(NB):
                    blocks.append(block(nb, 0, "q", qT_t))
                    blocks.append(block(nb, 1, "k", kT_t))
                return load_w, blocks, qT_t, kT_t

            load_w0, blocks0, qT0, kT0 = proj_pair_blocks(0)
            load_w0()

            def _pair0_emit(nb):
                blocks0[2 * nb]()
                blocks0[2 * nb + 1]()

            # ---- first pass over x: v projection (all heads) + pair-0 q/k ----
            pair0_hook = {"emit": _pair0_emit}
            with tc.tile_pool(name="vproj", bufs=1) as vpj:
              wv_t = vpj.tile([128, KD, INNER], F32R, tag="wv")
              wvr = wv.rearrange("(c p) i -> p c i", p=128)
              first_x = {}
              xTf = xT.rearrange("(c p) n -> p c n", p=128)
              for nb in range(NB):
                  xv_t = iopool.tile([128, KD, 512], F32R, tag="xv", bufs=2)
                  for dc in range(KD):
                      # interleave the wv chunks with the first x tile so the
                      # accumulation chain can start as soon as chunk 0 lands
                      if nb == 0:
                          nc.gpsimd.dma_start(wv_t[:, dc, :], wvr[:, dc, :])
                      nc.gpsimd.dma_start(
                          xv_t[:, dc, :],
                          xTf[:, dc, nb * 512 : (nb + 1) * 512],
                      )
                  first_x[nb] = xv_t
                  for sub in range(4):
                      pv = ps.tile([128, 512], F32, tag="s", bufs=3)
                      for dc in range(KD):
                          nc.tensor.matmul(
                              pv[:],
                              xv_t[:, dc, sub * 128 : (sub + 1) * 128],
                              wv_t[:, dc, :],
                              start=(dc == 0),
                              stop=(dc == KD - 1),
                          )
                      vstg = iopool.tile([128, 512], F32R, tag="vst", bufs=2)
                      nc.vector.tensor_copy(vstg[:], pv[:])
                      m0 = nb * 512 + sub * 128
                      nc.sync.dma_start(vs[m0 : m0 + 128, :], vstg[:])
                  pair0_hook["emit"](nb)
                  # write the already-cast x tile back to DRAM for the
                  # pair-1..3 projection passes (fast non-cast HWDGE)
                  nc.sync.dma_start(
                      xTr[:, :, nb * 512 : (nb + 1) * 512], xv_t[:]
                  )

            # pair-0 projections are emitted inside the first-pass loop via
            # pair0_hook (sharing its x tiles)
            pair_qk = {0: (qT0, kT0)}

            # ---- attention (pair p) interleaved with projections (p+1) ----
            with tc.tile_pool(name="attn", bufs=1) as at:
                otn = [
                    at.tile([128, 4, 512], F32R, tag=f"otn{p}", name=f"otn{p}")
                    for p in range(PAIRS)
                ]
                wo_h = []

                def load_wo():
                    for dh, wtag in ((0, "qT"), (1, "kT")):
                        woh = qkpool.tile(
                            [128, PAIRS, 512], F32R, tag=wtag, bufs=2,
                            name=f"wo_h{dh}",
                        )
                        nc.gpsimd.dma_start(
                            woh[:],
                            wo.rearrange("(c p) d -> p c d", p=128)[
                                :, :, dh * 512 : (dh + 1) * 512
                            ],
                        )
                        wo_h.append(woh)

                opq = []
                nmq = []

                def outproj_block(nb, dh):
                    def emit():
                        q4, r4 = divmod(nb, 4)
                        nsl = slice(nb * 128, (nb + 1) * 128)
                        po = ps.tile([128, 2, 512], F32, tag="s", bufs=3)
                        for c in range(PAIRS):
                            nc.tensor.matmul(
                                po[:, 0, :],
                                otn[c][:, q4, r4 * 128 : (r4 + 1) * 128],
                                wo_h[dh][:, c, :],
                                start=(c == 0),
                                stop=(c == PAIRS - 1),
                            )
                        ost = iopool.tile([128, 512], F32, tag="ost", bufs=2)
                        nc.any.tensor_copy(ost[:], po[:, 0, :])
                        nc.sync.dma_start(
                            out[nsl, dh * 512 : (dh + 1) * 512], ost[:]
                        )
                    return emit

                def outproj_quarter(q4):
                    # queue this quarter's out-projection; drained one block
                    # at a time inside the next quarter's attention loop
                    for r4 in range(4):
                        for dh in range(2):
                            opq.append(outproj_block(q4 * 4 + r4, dh))

                def load_vext(p):
                    ves = []
                    for j in range(2):
                        h = 2 * p + j
                        ve = at.tile(
                            [128, MB, 65], F32R, tag="vext", bufs=4,
                            name=f"ve_{p}_{j}",
                        )
                        nc.sync.dma_start(
                            ve[:, :, 0:64],
                            vs.rearrange("(mb q) i -> q mb i", q=128)[
                                :, :, h * 64 : (h + 1) * 64
                            ],
                        )
                        for mb in range(MB):
                            nc.gpsimd.tensor_copy(ve[:, mb, 64:65], onesr[:, 0:1])
                        ves.append(ve)
                    return ves

                vext_next = load_vext(0)
                for p in range(PAIRS):
                    qT_t, kT_t = pair_qk.pop(p)
                    vext = vext_next
                    if p == PAIRS - 1:
                        load_wo()
                    if p + 1 < PAIRS:
                        load_wn, blocks_n, qTn, kTn = proj_pair_blocks(p + 1)
                        load_wn()
                        pair_qk[p + 1] = (qTn, kTn)
                        vext_next = load_vext(p + 1)
                    else:
                        blocks_n = []
                    blk_i = 0
                    for f in range(2):
                        for sub in range(2):
                            n0 = f * 1024 + sub * 512
                            ot_ab = [
                                ps.tile([128, 512], F32, tag="ot", bufs=2, name=f"ot{jj}")
                                for jj in range(2)
                            ]
                            for mb2 in range(MB // 2):
                                s_tiles = []
                                for j in range(2):
                                    psl = slice(64 * j, 64 * (j + 1))
                                    s_t = ps.tile([128, 2, 512], F32, tag="s", bufs=3, name=f"s{j}")
                                    for hm in range(2):
                                        mb = 2 * mb2 + hm
                                        msl = slice(mb * 128, (mb + 1) * 128)
                                        nc.tensor.matmul(
                                            s_t[:, hm, :],
                                            kT_t[psl, msl],
                                            qT_t[psl, n0 : n0 + 512],
                                            start=True,
                                            stop=True,
                                        )
                                    s_tiles.append(s_t)
                                pts = []
                                for j in range(2):
                                    pt = at.tile([128, 2, 512], F32R, tag="pt", bufs=5, name=f"pt{j}")
                                    nc.scalar.activation(
                                        pt[:], s_tiles[j][:], EXP, scale=SCALE
                                    )
                                    pts.append(pt)
                                for j in range(2):
                                    for hm in range(2):
                                        mb = 2 * mb2 + hm
                                        nc.tensor.matmul(
                                            ot_ab[j][0:65, :],
                                            vext[j][:, mb, :],
                                            pts[j][:, hm, :],
                                            start=(mb == 0),
                                            stop=(mb == MB - 1),
                                        )
                                # previous quarter's deferred normalize: its
                                # reciprocal is long done by now, so the bcast
                                # MMs slot in without stalling. Both head
                                # halves must drain before any outproj pop
                                # below reads otn (write-after-read hazard).
                                if mb2 in (1, 2) and nmq:
                                    nmq.pop(0)()
                                # spread next pair's projection work through
                                # the attention chain to keep the PE dense
                                if mb2 % 2 == 1:
                                    if blk_i < len(blocks_n):
                                        blocks_n[blk_i]()
                                    blk_i += 1
                                    # in the last pair, spread the previous
                                    # quarter's output projection here too
                                    if mb2 >= 3:
                                        for _ in range(3):
                                            if opq:
                                                opq.pop(0)()
                            # spill OT accumulators to SBUF (frees the
                            # psum banks for the next quarter immediately)
                            osb = at.tile([65, 2, 512], F32, tag="ots", bufs=4)
                            nc.vector.tensor_copy(osb[:, 0, :], ot_ab[0][0:65, :])
                            nc.vector.tensor_copy(osb[:, 1, :], ot_ab[1][0:65, :])
                            # denominators -> approx recip (fp32) -> f32r cast;
                            # the bcast MMs + normalize muls are DEFERRED one
                            # quarter so the PE never waits on this DVE chain
                            rin = at.tile([33, 512], F32, tag="rin", bufs=2)
                            nc.vector.tensor_copy(rin[0:1, :], osb[64:65, 0, :])
                            nc.vector.tensor_copy(rin[32:33, :], osb[64:65, 1, :])
                            rec = at.tile([33, 512], F32R, tag="rec", bufs=2)
                            with nc.allow_low_precision(
                                reason="f32r reciprocal for softmax denom"
                            ):
                                # one op covers rows 0..32; rows 1-31 junk
                                nc.vector.reciprocal(rec[:], rin[:])

                            def norm_emit(j, p=p, f=f, sub=sub, osb=osb, rec=rec):
                                row = 32 * j
                                bc = ps.tile(
                                    [128, 2, 512], F32, tag="s", bufs=3,
                                    name=f"bc{j}",
                                )
                                nc.tensor.matmul(
                                    bc[0:64, 0, :],
                                    onesr[row : row + 1, :],
                                    rec[row : row + 1, :],
                                    start=True,
                                    stop=True,
                                )
                                nc.vector.tensor_mul(
                                    otn[p][64 * j : 64 * (j + 1), f * 2 + sub, :],
                                    osb[0:64, j, :],
                                    bc[0:64, 0, :],
                                )

                            nmq.append(lambda ne=norm_emit: ne(0))
                            nmq.append(lambda ne=norm_emit: ne(1))
                            if p == PAIRS - 1:
                                outproj_quarter(f * 2 + sub)
                while nmq:
                    nmq.pop(0)()
                while opq:
                    opq.pop(0)()

    return nc


_CACHED = {}


def _get_kernel():
    if "nc" not in _CACHED:
        _CACHED["nc"] = build_kernel()
    return _CACHED["nc"]


def kernel(x, rotary_emb_x, Wq, Wkv, Wo, bo):
    from concourse.bass_utils import run_bass_kernel_spmd

    x = np.asarray(x, np.float32)
    rope = np.asarray(rotary_emb_x, np.float32)
    Wq = np.asarray(Wq, np.float32)
    Wkv = np.asarray(Wkv, np.float32)
    Wo = np.asarray(Wo, np.float32)
    bo = np.asarray(bo, np.float32)

    cosT = np.ascontiguousarray(np.cos(rope).T)  # [64, N]
    sinT = np.ascontiguousarray(np.sin(rope).T)
    cosT2 = np.ascontiguousarray(np.concatenate([cosT, cosT], axis=0))
    sinT2 = np.concatenate([sinT, sinT], axis=0)
    # fold rotate_half's sign into sin: the low half of each 64-row head
    # block multiplies -q_hi
    sinT2 = sinT2.copy()
    sinT2[0:32] = -sinT2[0:32]
    sinT2[64:96] = -sinT2[64:96]
    sinT2 = np.ascontiguousarray(sinT2)

    Wk_full = Wkv[:, : H * DH]
    Wv_full = Wkv[:, H * DH :]

    xTs = [np.ascontiguousarray(x[b].T) for b in range(B)]
    in_maps = []
    for core in range(N_CORES):
        b, hg = divmod(core, 2)
        isl = slice(hg * INNER, (hg + 1) * INNER)
        in_maps.append(
            {
                "xT": xTs[b],
                "wq": np.ascontiguousarray(Wq[:, isl]),
                "wk": np.ascontiguousarray(Wk_full[:, isl]),
                "wv": np.ascontiguousarray(Wv_full[:, isl]),
                "wo": np.ascontiguousarray(Wo[isl, :]),
                "cosT": cosT2,
                "sinT": sinT2,
            }
        )

    nc = _get_kernel()
    _CACHED["in_maps"] = in_maps
    res = run_bass_kernel_spmd(nc, in_maps, list(range(N_CORES)))
    outs = [res.results[i]["out"] for i in range(N_CORES)]
    full = np.stack(
        [outs[2 * b] + outs[2 * b + 1] + bo for b in range(B)], axis=0
    )
    return full

